# revision 58
# baseline (speedup 1.0000x reference)
"""Trainium2 Bass kernel for multi-head attention (B=4, N=2048, C=768, H=12).

Sharding: zero-collective data parallel across 8 NeuronCores. Core c handles
batch b=c//2 and query rows [(c%2)*1024, +1024). K/V are computed over all
2048 keys of the batch (softmax over keys is permutation invariant, so each
core receives its x with its own query rows rolled to the front). Pairs of
cores duplicate only the cheap K/V projection; there are no collectives.

Key performance facts (measured on hw): PE matmuls with contraction K=128
run at ~227ns/512 rows (2.4GHz); K<128 can silently run at HALF rate, so
the 65-row q/k tiles (64 dims + 1 shift lane) are zero-padded to 128
partitions (zeros DMA'd from DRAM; padding contributes 0 to the product).

Per-core pipeline (matmul compute dtype float32r ~ tf32; rel err ~2.8e-3
against the 2e-2 gate):
  P0: load x [2048,768] in 8 chunks alternating the SP/Act DMA queues,
      PE-transpose to xT [768,2048] (f32r rounding in the PSUM->SBUF copies).
  P1: QKV projections, w_qkv streamed in six 384-col chunks (bufs=2) so
      weight DMA+cast overlap the matmuls; kc-outer/ncn-inner loop order
      loads each weight tile once. q-tilde [128,1024] per head (row 64 =
      +row-max, rows 65+ zero), k-tilde (row 64 = -1) staged through DRAM,
      v-hat [128,16,h,65] bf16 (col 64 = ones so A^T.v-hat also yields the
      softmax denominators). w_proj is loaded/cast here too.
  P2: software-pipelined TWO heads deep. During the tail of head h, one
      S-pass step of head h+2 runs per iteration, so the row-max bounce DMA
      (PSUM row-max on DVE -> transposing DRAM bounce into q-tilde row 64)
      has the whole of tail h+1 to land. Tail iteration: S~^T pair (the
      65th lane applies -max) -> exp(8*x) on ScalarE -> bf16 A^T; the AV
      pair of mt-1 issues after the ST pair of mt so the in-order PE queue
      never waits on the Act engine. Normalization: reciprocal_approx_fast
      (denominators are in [1, 2048] -> safe) + gpsimd partition_broadcast
      + DVE multiply into outNT [768,1024] f32r.
  P3: projection out = outNT.T @ w_proj (outNT is directly the lhsT),
      kc-outer loop order, DMA out [1024,768].
"""

import numpy as np

NQ = 1024  # queries per core
NK = 2048  # keys per core
CD = 768
H = 12
D = 64
P = 128

_CACHE = {}


def _build():
    from contextlib import ExitStack

    import concourse.bacc as bacc
    import concourse.mybir as mybir
    import concourse.tile as tile

    f32 = mybir.dt.float32
    f32r = mybir.dt.float32r
    bf16 = mybir.dt.bfloat16
    EXP = mybir.ActivationFunctionType.Exp
    LOG = mybir.ActivationFunctionType.Ln

    nc = bacc.Bacc("TRN2", target_bir_lowering=False, debug=False, num_devices=8)

    x_ap = nc.dram_tensor("x", [NK, CD], f32, kind="ExternalInput").ap()
    wqkv_ap = nc.dram_tensor("w_qkv", [CD, 3 * CD], f32, kind="ExternalInput").ap()
    wproj_ap = nc.dram_tensor("w_proj", [CD, CD], f32, kind="ExternalInput").ap()
    ident_ap = nc.dram_tensor("ident", [P, P], f32, kind="ExternalInput").ap()
    # crow[0,:] = 0.0 (q-tilde row 64 init), crow[1,:] = -1.0 (k-tilde row 64)
    crow_ap = nc.dram_tensor("crow", [2, NK], f32r, kind="ExternalInput").ap()
    out_ap = nc.dram_tensor("out", [NQ, CD], f32, kind="ExternalOutput").ap()

    # zero padding rows 65..127 of q/k tiles: K=128 matmuls run 2x faster
    # than K=65 on the PE (empirically; K<128 can drop to half rate)
    zeros_ap = nc.dram_tensor("zeros", [P - D - 1, NK], f32r, kind="ExternalInput").ap()

    kstg_ap = nc.dram_tensor("kstg", [H, D + 1, NK], f32r).ap()
    mstg_ap = nc.dram_tensor("mstg", [H, NQ], f32r).ap()

    x_t = x_ap.rearrange("(t p) c -> p t c", p=P)  # [128, 16, 768]
    wqkv_t = wqkv_ap.rearrange("(a p) n -> p a n", p=P)  # [128, 6, 2304]
    wproj_t = wproj_ap.rearrange("(a p) n -> p a n", p=P)  # [128, 6, 768]
    out_t = out_ap.rearrange("(t p) c -> p t c", p=P)  # [128, 8, 768]

    with tile.TileContext(nc) as tc, ExitStack() as ctx:
        # ---- persistent pools ----
        pers = ctx.enter_context(tc.tile_pool(name="pers", bufs=1))
        # padded so downstream pools (xT, q-tiles) start 4KB-aligned — matmuls
        # whose moving operand is misaligned from 1KB run ~57% slower
        ident_sb = pers.tile([P, P], f32, tag="ident", padded_shape=[P, 1024])
        nc.sync.dma_start(ident_sb[:], ident_ap)

        qpool = ctx.enter_context(tc.tile_pool(name="qt", bufs=1))
        q_tiles = [
            qpool.tile([P, NQ], f32r, tag=f"q{h}", name=f"q{h}") for h in range(H)
        ]

        vpool = ctx.enter_context(tc.tile_pool(name="vhat", bufs=1))
        vhat = vpool.tile([P, 16, H, D + 1], bf16, tag="vhat")

        wpp = ctx.enter_context(tc.tile_pool(name="wp", bufs=1))
        wp_r = wpp.tile([P, 6, CD], f32r, tag="wp")

        from contextlib import ExitStack as _ES

        ctx_xT = _ES()
        xT_pool = ctx_xT.enter_context(tc.tile_pool(name="xT", bufs=1))
        xT = xT_pool.tile([P, 6, NK], f32r, tag="xT")  # [C-chunk part, kc, row]

        # ================= P0: load x, transpose =================
        with (
            tc.tile_pool(name="xn", bufs=1) as xnp,
            tc.tile_pool(name="ps0", bufs=4, space="PSUM") as ps0,
        ):
            xn = xnp.tile([P, 16, CD], f32, tag="xn")
            for tq in range(16):
                eng = nc.sync if tq % 2 == 0 else nc.scalar
                eng.dma_start(xn[:, tq : tq + 1, :], x_t[:, tq : tq + 1, :])
            # q-tile zero padding, queued after the x chunks; only needed
            # before P2's S-pass
            for h in range(H):
                eng = nc.sync if h % 2 == 0 else nc.scalar
                eng.dma_start(q_tiles[h][D + 1 : P, :], zeros_ap[:, 0:NQ])
            for t in range(16):
                for kc in range(6):
                    pst = ps0.tile([P, P], f32, tag="tr")
                    nc.tensor.transpose(
                        pst[:], xn[:, t, kc * P : (kc + 1) * P], ident_sb[:]
                    )
                    nc.vector.tensor_copy(
                        xT[:, kc, t * P : (t + 1) * P], pst[:]
                    )  # f32 -> f32r round

        # ================= P1: QKV projections =================
        # w_qkv streamed in six 384-col chunks (bufs=2) so weight DMA+cast of
        # chunk c+1 overlaps the matmuls of chunk c
        def qk_col_block(j, wq_r, jl, is_q):
            """One 128-col block (heads 2j, 2j+1) of the q or k projection."""
            nch = 2 if is_q else 4
            if not is_q:
                dsts = [
                    kstg_pool.tile(
                        [D + 1, NK], f32r, tag="kst", name=f"kst{2 * j + i}"
                    )
                    for i in range(2)
                ]
            else:
                dsts = [q_tiles[2 * j], q_tiles[2 * j + 1]]
            # kc outer / ncn inner: each weight tile is loaded once and used
            # for all ncn chunks (the PE skips nothing, but redundant
            # LDWEIGHTS pressure goes away)
            ps_list = [ps1.tile([P, 512], f32, tag="qk", name=f"qk{_n}") for _n in range(nch)]
            for kc in range(6):
                for ncn in range(nch):
                    nc.tensor.matmul(
                        ps_list[ncn][:],
                        wq_r[:, kc, jl * P : (jl + 1) * P],
                        xT[:, kc, ncn * 512 : (ncn + 1) * 512],
                        start=(kc == 0),
                        stop=(kc == 5),
                    )
            for ncn in range(nch):
                for i in range(2):
                    nc.scalar.copy(
                        dsts[i][0:D, ncn * 512 : (ncn + 1) * 512],
                        ps_list[ncn][i * D : (i + 1) * D, :],
                    )
            for i in range(2):
                if is_q:
                    nc.sync.dma_start(dsts[i][D : D + 1, 0:NQ], crow_ap[0, 0:NQ])
                else:
                    nc.sync.dma_start(dsts[i][D : D + 1, :], crow_ap[1, :])
                    nc.sync.dma_start(kstg_ap[2 * j + i], dsts[i][:])

        with (
            tc.tile_pool(name="wq", bufs=2) as wqp,
            tc.tile_pool(name="wtmp", bufs=2) as wtp,
            tc.tile_pool(name="kstg", bufs=4) as kstg_pool,
            tc.tile_pool(name="ps1", bufs=4, space="PSUM") as ps1,
            tc.tile_pool(name="ps1v", bufs=2, space="PSUM") as ps1v,
        ):
            def load_chunk(c):
                wq_r = wqp.tile([P, 6, 384], f32r, tag="wq")
                for a in range(6):
                    wtmp = wtp.tile([P, 384], f32, tag="wt")
                    eng = nc.sync if a % 2 == 0 else nc.scalar
                    eng.dma_start(wtmp[:], wqkv_t[:, a, 384 * c : 384 * (c + 1)])
                    nc.vector.tensor_copy(wq_r[:, a, :], wtmp[:])
                return wq_r

            wq_r0 = load_chunk(0)
            # w_proj load queued after the first chunk so it never gates P1
            for a in range(6):
                wtmp3t = wtp.tile([P, CD], f32, tag="wt3")
                eng = nc.sync if a % 2 == 0 else nc.scalar
                eng.dma_start(wtmp3t[:], wproj_t[:, a, :])
                nc.vector.tensor_copy(wp_r[:, a, :], wtmp3t[:])
            for c in range(2):  # q cols 0..767
                wq_r = wq_r0 if c == 0 else load_chunk(c)
                for jl in range(3):
                    qk_col_block(3 * c + jl, wq_r, jl, is_q=True)
            for c in range(2, 4):  # k cols 768..1535
                wq_r = load_chunk(c)
                for jl in range(3):
                    qk_col_block(3 * (c - 2) + jl, wq_r, jl, is_q=False)
            for c in range(4, 6):  # v cols 1536..2303
                wq_r = load_chunk(c)
                vc = c - 4
                for t in range(16):
                    ps = ps1v.tile([P, 384], f32, tag="vp")
                    for kc in range(6):
                        nc.tensor.matmul(
                            ps[:],
                            xT[:, kc, t * P : (t + 1) * P],
                            wq_r[:, kc, :],
                            start=(kc == 0),
                            stop=(kc == 5),
                        )
                    nc.vector.tensor_copy(
                        vhat[:, t, 6 * vc : 6 * (vc + 1), 0:D],
                        ps[:].rearrange("p (h d) -> p h d", d=D),
                    )
            nc.gpsimd.memset(vhat[:, :, :, D], 1.0)
        ctx_xT.close()  # xT dead after P1 — free 48KB/partition for P2

        # ================= P2: attention per head =================
        opool = ctx.enter_context(tc.tile_pool(name="outNT", bufs=1))
        outNT = opool.tile([P, 6, NQ], f32r, tag="outNT")
        with (
            tc.tile_pool(name="ka", bufs=3) as kap,
            tc.tile_pool(name="mx", bufs=2) as mxp,
            tc.tile_pool(name="at", bufs=4) as atp,
            tc.tile_pool(name="nrm", bufs=2) as nrmp,
            tc.tile_pool(name="scr", bufs=2) as scrp,
            tc.tile_pool(name="psS", bufs=2, space="PSUM") as psS,
            tc.tile_pool(name="psST", bufs=2, space="PSUM") as psST,
            tc.tile_pool(name="psAV", bufs=2, space="PSUM") as psAV,
        ):
            ka_tiles = {}
            max2_tiles = {}

            # zero rows 65..127 of the three rotating ka buffers once; later
            # generations only DMA rows 0..64, the pad rows stay zero forever
            for b in range(3):
                kaz = kap.tile([P, NK], f32r, tag="ka", name=f"kaz{b}")
                eng = nc.sync if b % 2 == 0 else nc.scalar
                eng.dma_start(kaz[D + 1 : P, :], zeros_ap)

            def s_begin(h):
                ka = kap.tile([P, NK], f32r, tag="ka", name=f"ka{h}")
                ka_tiles[h] = ka
                nc.sync.dma_start(ka[0 : D + 1, :], kstg_ap[h])
                max2_tiles[h] = mxp.tile([P, 16], f32, tag="mx2", name=f"mx2{h}")

            def s_step(h, i):
                # one [128,1024] S tile: queries qt=i//2, keys half2=i%2.
                # tensor_tensor_reduce fuses pairwise-max of the two 512-col
                # halves with the row-max reduction in a single DVE pass
                qa, ka, max2 = q_tiles[h], ka_tiles[h], max2_tiles[h]
                qt, half2 = i // 2, i % 2
                ps_s = psS.tile([P, NK // 2], f32, tag="S")
                for mc in range(2):
                    m0 = half2 * 1024 + mc * 512
                    nc.tensor.matmul(
                        ps_s[:, mc * 512 : (mc + 1) * 512],
                        qa[:, qt * P : (qt + 1) * P],
                        ka[:, m0 : m0 + 512],
                        start=True,
                        stop=True,
                    )
                nc.vector.reduce_max(
                    max2[:, 2 * qt + half2 : 2 * qt + half2 + 1],
                    ps_s[:],
                    axis=mybir.AxisListType.X,
                )

            def s_end(h):
                qa, max2 = q_tiles[h], max2_tiles[h]
                maxsb = mxp.tile([P, 8], f32, tag="mx", name=f"mx{h}")
                nc.vector.tensor_tensor(
                    out=maxsb[:],
                    in0=max2[:, 0:16:2],
                    in1=max2[:, 1:16:2],
                    op=mybir.AluOpType.max,
                )
                maxr = mxp.tile([P, 8], f32r, tag="mxr", name=f"mxr{h}")
                nc.vector.tensor_copy(maxr[:], maxsb[:])
                # scatter maxes into q-tilde row 64 (elem (p,qt) -> col qt*128+p)
                # via a transposing DRAM bounce
                nc.sync.dma_start(mstg_ap[h].rearrange("(a b) -> b a", b=P), maxr[:])
                nc.sync.dma_start(qa[D : D + 1, :], mstg_ap[h])

            av_tiles = {}

            def t_begin(h):
                av_tiles[h] = [
                    psAV.tile([D + 1, 512], f32, tag="av", name=f"av{h}_{i}")
                    for i in range(2)
                ]

            at_tiles = {}

            def t_st(h, mt):
                # both ncn S~^T matmuls + exps; independent of the AV pair of
                # mt-1 that issues after them (deferred one iteration so the
                # PE never waits on the Act engine)
                qa, ka = q_tiles[h], ka_tiles[h]
                ats = []
                for ncn in range(2):
                    ps_st = psST.tile([P, 512], f32, tag="st")
                    nc.tensor.matmul(
                        ps_st[:],
                        ka[:, mt * P : (mt + 1) * P],
                        qa[:, ncn * 512 : (ncn + 1) * 512],
                        start=True,
                        stop=True,
                    )
                    at = atp.tile([P, 512], bf16, tag="at")
                    nc.scalar.activation(at[:], ps_st[:], EXP, scale=8.0)
                    ats.append(at)
                at_tiles[(h, mt)] = ats

            def t_av(h, mt):
                ats = at_tiles.pop((h, mt))
                for ncn in range(2):
                    nc.tensor.matmul(
                        av_tiles[h][ncn][:],
                        vhat[:, mt, h, :],
                        ats[ncn][:],
                        start=(mt == 0),
                        stop=(mt == 15),
                    )

            def t_end(h):
                for ncn in range(2):
                    ps_av = av_tiles[h][ncn]
                    den = nrmp.tile([1, 512], f32, tag="den")
                    nc.scalar.copy(den[:], ps_av[D : D + 1, :])
                    rsb = nrmp.tile([1, 512], f32, tag="rec")
                    nc.vector.reciprocal_approx_fast(rsb[:], den[:])
                    rb = nrmp.tile([D, 512], f32, tag="rb")
                    nc.gpsimd.partition_broadcast(rb[:], rsb[:])
                    r0 = D * (h % 2)
                    nc.vector.tensor_mul(
                        outNT[r0 : r0 + D, h // 2, ncn * 512 : (ncn + 1) * 512],
                        ps_av[0:D, :],
                        rb[:],
                    )

            # software pipeline, two heads deep: during tail h the S-pass of
            # head h+2 runs one step per iteration, so its max-bounce DMA has
            # the whole of tail h+1 to land before tail h+2 needs the maxes.
            # AV of mt issues a full iteration after its exp started, so the
            # PE never stalls on the Act engine.
            s_begin(0)
            for i in range(16):
                s_step(0, i)
            s_end(0)
            # tail(0) carries the S-passes of BOTH heads 1 and 2 (two steps
            # per iteration; head 1 in the first half with its s_end mid-loop
            # so the max-bounce launches as soon as the reduces drain)
            for h in range(H):
                t_begin(h)
                if h == 0:
                    s_begin(1)
                    s_begin(2)
                elif h + 2 < H:
                    s_begin(h + 2)
                for mt in range(16):
                    t_st(h, mt)
                    if h == 0:
                        if mt < 8:
                            s_step(1, 2 * mt)
                            s_step(1, 2 * mt + 1)
                        else:
                            s_step(2, 2 * (mt - 8))
                            s_step(2, 2 * (mt - 8) + 1)
                        if mt == 7:
                            s_end(1)
                    elif h + 2 < H:
                        s_step(h + 2, mt)
                    if mt >= 1:
                        t_av(h, mt - 1)
                t_av(h, 15)
                t_end(h)
                if h == 0:
                    s_end(2)
                elif h + 2 < H:
                    s_end(h + 2)

        # ================= P3: output projection =================
        with (
            tc.tile_pool(name="ob", bufs=2) as obp,
            tc.tile_pool(name="psP", bufs=2, space="PSUM") as psP,
        ):
            for nt in range(8):
                ps = psP.tile([P, CD], f32, tag="pj")
                for kc in range(6):
                    for c0, cn in ((0, 512), (512, 256)):
                        nc.tensor.matmul(
                            ps[:, c0 : c0 + cn],
                            outNT[:, kc, nt * P : (nt + 1) * P],
                            wp_r[:, kc, c0 : c0 + cn],
                            start=(kc == 0),
                            stop=(kc == 5),
                        )
                osb = obp.tile([P, CD], f32, tag="ob")
                nc.scalar.copy(osb[:], ps[:])
                oeng = nc.sync if nt % 2 == 0 else nc.scalar
                oeng.dma_start(out_t[:, nt, :], osb[:])

    nc.compile()
    return nc


def _in_maps(x, w_qkv, w_proj):
    ident = np.eye(P, dtype=np.float32)
    crow = np.zeros((2, NK), np.float32)
    crow[1, :] = -1.0
    zeros = np.zeros((P - D - 1, NK), np.float32)
    maps = []
    for c in range(8):
        b, qh = c // 2, c % 2
        xb = np.roll(x[b], -qh * NQ, axis=0) if qh else x[b]
        maps.append(
            {
                "x": np.ascontiguousarray(xb, dtype=np.float32),
                "w_qkv": np.ascontiguousarray(w_qkv, dtype=np.float32),
                "w_proj": np.ascontiguousarray(w_proj, dtype=np.float32),
                "ident": ident,
                "crow": crow,
                "zeros": zeros,
            }
        )
    return maps


LAST = {}


def kernel(x, w_qkv, w_proj):
    import os

    from concourse import bass_utils

    if "nc" not in _CACHE:
        _CACHE["nc"] = _build()
    nc = _CACHE["nc"]
    kwargs = {}
    if os.environ.get("KERNEL_TRACE"):
        kwargs["trace"] = True
        if os.environ.get("KERNEL_TRACE_DIR"):
            kwargs["tmpdir"] = os.environ["KERNEL_TRACE_DIR"]
    res = bass_utils.run_bass_kernel_spmd(
        nc, _in_maps(x, w_qkv, w_proj), core_ids=list(range(8)), **kwargs
    )
    LAST["exec_time_ns"] = res.exec_time_ns
    out = np.empty((4, 2048, CD), np.float32)
    for c in range(8):
        b, qh = c // 2, c % 2
        out[b, qh * NQ : (qh + 1) * NQ] = res.results[c]["out"]
    return out



# revision 59
# speedup vs baseline: 1.0252x; 1.0252x over previous
"""Trainium2 Bass kernel for multi-head attention (B=4, N=2048, C=768, H=12).

Sharding: zero-collective data parallel across 8 NeuronCores. Core c handles
batch b=c//2 and query rows [(c%2)*1024, +1024). K/V are computed over all
2048 keys of the batch (softmax over keys is permutation invariant, so each
core receives its x with its own query rows rolled to the front). Pairs of
cores duplicate only the cheap K/V projection; there are no collectives.

Key performance facts (measured on hw): PE matmuls with contraction K=128
run at ~227ns/512 rows (2.4GHz); K<128 can silently run at HALF rate, so
the 65-row q/k tiles (64 dims + 1 shift lane) are zero-padded to 128
partitions (zeros DMA'd from DRAM; padding contributes 0 to the product).

Per-core pipeline (matmul compute dtype float32r ~ tf32; rel err ~2.8e-3
against the 2e-2 gate):
  P0: load x [2048,768] in 8 chunks alternating the SP/Act DMA queues,
      PE-transpose to xT [768,2048] (f32r rounding in the PSUM->SBUF copies).
  P1: QKV projections, w_qkv streamed in six 384-col chunks (bufs=2) so
      weight DMA+cast overlap the matmuls; kc-outer/ncn-inner loop order
      loads each weight tile once. q-tilde [128,1024] per head (row 64 =
      +row-max, rows 65+ zero), k-tilde (row 64 = -1) staged through DRAM,
      v-hat [128,16,h,65] bf16 (col 64 = ones so A^T.v-hat also yields the
      softmax denominators). w_proj is loaded/cast here too.
  P2: software-pipelined TWO heads deep. During the tail of head h, one
      S-pass step of head h+2 runs per iteration, so the row-max bounce DMA
      (PSUM row-max on DVE -> transposing DRAM bounce into q-tilde row 64)
      has the whole of tail h+1 to land. Tail iteration: S~^T pair (the
      65th lane applies -max) -> exp(8*x) on ScalarE -> bf16 A^T; the AV
      pair of mt-1 issues after the ST pair of mt so the in-order PE queue
      never waits on the Act engine. Normalization: reciprocal_approx_fast
      (denominators are in [1, 2048] -> safe) + gpsimd partition_broadcast
      + DVE multiply into outNT [768,1024] f32r.
  P3: projection out = outNT.T @ w_proj (outNT is directly the lhsT),
      kc-outer loop order, DMA out [1024,768].
"""

import numpy as np

NQ = 1024  # queries per core
NK = 2048  # keys per core
CD = 768
H = 12
D = 64
P = 128

_CACHE = {}


def _build():
    from contextlib import ExitStack

    import concourse.bacc as bacc
    import concourse.mybir as mybir
    import concourse.tile as tile

    f32 = mybir.dt.float32
    f32r = mybir.dt.float32r
    bf16 = mybir.dt.bfloat16
    EXP = mybir.ActivationFunctionType.Exp
    LOG = mybir.ActivationFunctionType.Ln

    nc = bacc.Bacc("TRN2", target_bir_lowering=False, debug=False, num_devices=8)

    x_ap = nc.dram_tensor("x", [NK, CD], f32, kind="ExternalInput").ap()
    wqkv_ap = nc.dram_tensor("w_qkv", [CD, 3 * CD], f32, kind="ExternalInput").ap()
    wproj_ap = nc.dram_tensor("w_proj", [CD, CD], f32, kind="ExternalInput").ap()
    ident_ap = nc.dram_tensor("ident", [P, P], f32, kind="ExternalInput").ap()
    # crow[0,:] = 0.0 (q-tilde row 64 init), crow[1,:] = -1.0 (k-tilde row 64)
    crow_ap = nc.dram_tensor("crow", [2, NK], f32r, kind="ExternalInput").ap()
    out_ap = nc.dram_tensor("out", [NQ, CD], f32, kind="ExternalOutput").ap()

    # zero padding rows 65..127 of q/k tiles: K=128 matmuls run 2x faster
    # than K=65 on the PE (empirically; K<128 can drop to half rate)
    zeros_ap = nc.dram_tensor("zeros", [P - D - 1, NK], f32r, kind="ExternalInput").ap()

    kstg_ap = nc.dram_tensor("kstg", [H, D + 1, NK], f32r).ap()
    mstg_ap = nc.dram_tensor("mstg", [H, NQ], f32r).ap()

    x_t = x_ap.rearrange("(t p) c -> p t c", p=P)  # [128, 16, 768]
    wqkv_t = wqkv_ap.rearrange("(a p) n -> p a n", p=P)  # [128, 6, 2304]
    wproj_t = wproj_ap.rearrange("(a p) n -> p a n", p=P)  # [128, 6, 768]
    out_t = out_ap.rearrange("(t p) c -> p t c", p=P)  # [128, 8, 768]

    with tile.TileContext(nc) as tc, ExitStack() as ctx:
        # ---- persistent pools ----
        pers = ctx.enter_context(tc.tile_pool(name="pers", bufs=1))
        # padded so downstream pools (xT, q-tiles) start 4KB-aligned — matmuls
        # whose moving operand is misaligned from 1KB run ~57% slower
        ident_sb = pers.tile([P, P], f32, tag="ident", padded_shape=[P, 1024])
        nc.sync.dma_start(ident_sb[:], ident_ap)

        qpool = ctx.enter_context(tc.tile_pool(name="qt", bufs=1))
        q_tiles = [
            qpool.tile([P, NQ], f32r, tag=f"q{h}", name=f"q{h}") for h in range(H)
        ]

        vpool = ctx.enter_context(tc.tile_pool(name="vhat", bufs=1))
        vhat = vpool.tile([P, 16, H, D + 1], bf16, tag="vhat")

        wpp = ctx.enter_context(tc.tile_pool(name="wp", bufs=1))
        wp_r = wpp.tile([P, 6, CD], f32r, tag="wp")

        from contextlib import ExitStack as _ES

        ctx_xT = _ES()
        xT_pool = ctx_xT.enter_context(tc.tile_pool(name="xT", bufs=1))
        xT = xT_pool.tile([P, 6, NK], f32r, tag="xT")  # [C-chunk part, kc, row]

        # ================= P0: load x, transpose =================
        with (
            tc.tile_pool(name="xn", bufs=1) as xnp,
            tc.tile_pool(name="ps0", bufs=4, space="PSUM") as ps0,
        ):
            xn = xnp.tile([P, 16, CD], f32, tag="xn")
            for tq in range(16):
                eng = nc.sync if tq % 2 == 0 else nc.scalar
                eng.dma_start(xn[:, tq : tq + 1, :], x_t[:, tq : tq + 1, :])
            # q-tile zero padding, queued after the x chunks; only needed
            # before P2's S-pass
            for h in range(H):
                eng = nc.sync if h % 2 == 0 else nc.scalar
                eng.dma_start(q_tiles[h][D + 1 : P, :], zeros_ap[:, 0:NQ])
            for t in range(16):
                for kc in range(6):
                    pst = ps0.tile([P, P], f32, tag="tr")
                    nc.tensor.transpose(
                        pst[:], xn[:, t, kc * P : (kc + 1) * P], ident_sb[:]
                    )
                    nc.vector.tensor_copy(
                        xT[:, kc, t * P : (t + 1) * P], pst[:]
                    )  # f32 -> f32r round

        # ================= P1: QKV projections =================
        # w_qkv streamed in six 384-col chunks (bufs=2) so weight DMA+cast of
        # chunk c+1 overlaps the matmuls of chunk c
        def qk_col_block(j, wq_r, jl, is_q):
            """One 128-col block (heads 2j, 2j+1) of the q or k projection."""
            nch = 2 if is_q else 4
            if not is_q:
                dsts = [
                    kstg_pool.tile(
                        [D + 1, NK], f32r, tag="kst", name=f"kst{2 * j + i}"
                    )
                    for i in range(2)
                ]
            else:
                dsts = [q_tiles[2 * j], q_tiles[2 * j + 1]]
            # kc outer / ncn inner: each weight tile is loaded once and used
            # for all ncn chunks (the PE skips nothing, but redundant
            # LDWEIGHTS pressure goes away)
            ps_list = [ps1.tile([P, 512], f32, tag="qk", name=f"qk{_n}") for _n in range(nch)]
            for kc in range(6):
                for ncn in range(nch):
                    nc.tensor.matmul(
                        ps_list[ncn][:],
                        wq_r[:, kc, jl * P : (jl + 1) * P],
                        xT[:, kc, ncn * 512 : (ncn + 1) * 512],
                        start=(kc == 0),
                        stop=(kc == 5),
                    )
            for ncn in range(nch):
                # split the PSUM->SBUF copies between Act and DVE: both have
                # slack in P1 and the copies gate the k-staging writes
                nc.scalar.copy(
                    dsts[0][0:D, ncn * 512 : (ncn + 1) * 512],
                    ps_list[ncn][0:D, :],
                )
                nc.vector.tensor_copy(
                    dsts[1][0:D, ncn * 512 : (ncn + 1) * 512],
                    ps_list[ncn][D : 2 * D, :],
                )
            for i in range(2):
                if is_q:
                    nc.sync.dma_start(dsts[i][D : D + 1, 0:NQ], crow_ap[0, 0:NQ])
                else:
                    nc.sync.dma_start(dsts[i][D : D + 1, :], crow_ap[1, :])
                    nc.sync.dma_start(kstg_ap[2 * j + i], dsts[i][:])

        with (
            tc.tile_pool(name="wq", bufs=2) as wqp,
            tc.tile_pool(name="wtmp", bufs=2) as wtp,
            tc.tile_pool(name="kstg", bufs=4) as kstg_pool,
            tc.tile_pool(name="ps1", bufs=4, space="PSUM") as ps1,
            tc.tile_pool(name="ps1v", bufs=2, space="PSUM") as ps1v,
        ):
            def load_chunk(c):
                wq_r = wqp.tile([P, 6, 384], f32r, tag="wq")
                for a in range(6):
                    wtmp = wtp.tile([P, 384], f32, tag="wt")
                    eng = nc.sync if a % 2 == 0 else nc.scalar
                    eng.dma_start(wtmp[:], wqkv_t[:, a, 384 * c : 384 * (c + 1)])
                    nc.vector.tensor_copy(wq_r[:, a, :], wtmp[:])
                return wq_r

            wq_r0 = load_chunk(0)
            # w_proj load queued after the first chunk so it never gates P1
            for a in range(6):
                wtmp3t = wtp.tile([P, CD], f32, tag="wt3")
                eng = nc.sync if a % 2 == 0 else nc.scalar
                eng.dma_start(wtmp3t[:], wproj_t[:, a, :])
                nc.vector.tensor_copy(wp_r[:, a, :], wtmp3t[:])
            for c in range(2):  # q cols 0..767
                wq_r = wq_r0 if c == 0 else load_chunk(c)
                for jl in range(3):
                    qk_col_block(3 * c + jl, wq_r, jl, is_q=True)
            for c in range(2, 4):  # k cols 768..1535
                wq_r = load_chunk(c)
                for jl in range(3):
                    qk_col_block(3 * (c - 2) + jl, wq_r, jl, is_q=False)
            for c in range(4, 6):  # v cols 1536..2303
                wq_r = load_chunk(c)
                vc = c - 4
                for t in range(16):
                    ps = ps1v.tile([P, 384], f32, tag="vp")
                    for kc in range(6):
                        nc.tensor.matmul(
                            ps[:],
                            xT[:, kc, t * P : (t + 1) * P],
                            wq_r[:, kc, :],
                            start=(kc == 0),
                            stop=(kc == 5),
                        )
                    nc.vector.tensor_copy(
                        vhat[:, t, 6 * vc : 6 * (vc + 1), 0:D],
                        ps[:].rearrange("p (h d) -> p h d", d=D),
                    )
            nc.gpsimd.memset(vhat[:, :, :, D], 1.0)
        ctx_xT.close()  # xT dead after P1 — free 48KB/partition for P2

        # ================= P2: attention per head =================
        opool = ctx.enter_context(tc.tile_pool(name="outNT", bufs=1))
        outNT = opool.tile([P, 6, NQ], f32r, tag="outNT")
        with (
            tc.tile_pool(name="ka", bufs=3) as kap,
            tc.tile_pool(name="mx", bufs=2) as mxp,
            tc.tile_pool(name="at", bufs=4) as atp,
            tc.tile_pool(name="nrm", bufs=2) as nrmp,
            tc.tile_pool(name="scr", bufs=2) as scrp,
            tc.tile_pool(name="psS", bufs=2, space="PSUM") as psS,
            tc.tile_pool(name="psST", bufs=2, space="PSUM") as psST,
            tc.tile_pool(name="psAV", bufs=2, space="PSUM") as psAV,
        ):
            ka_tiles = {}
            max2_tiles = {}

            # zero rows 65..127 of the three rotating ka buffers once; later
            # generations only DMA rows 0..64, the pad rows stay zero forever
            for b in range(3):
                kaz = kap.tile([P, NK], f32r, tag="ka", name=f"kaz{b}")
                eng = nc.sync if b % 2 == 0 else nc.scalar
                eng.dma_start(kaz[D + 1 : P, :], zeros_ap)

            def s_begin(h):
                ka = kap.tile([P, NK], f32r, tag="ka", name=f"ka{h}")
                ka_tiles[h] = ka
                nc.sync.dma_start(ka[0 : D + 1, :], kstg_ap[h])
                max2_tiles[h] = mxp.tile([P, 16], f32, tag="mx2", name=f"mx2{h}")

            def s_step(h, i):
                # one [128,1024] S tile: queries qt=i//2, keys half2=i%2.
                # tensor_tensor_reduce fuses pairwise-max of the two 512-col
                # halves with the row-max reduction in a single DVE pass
                qa, ka, max2 = q_tiles[h], ka_tiles[h], max2_tiles[h]
                qt, half2 = i // 2, i % 2
                ps_s = psS.tile([P, NK // 2], f32, tag="S")
                for mc in range(2):
                    m0 = half2 * 1024 + mc * 512
                    nc.tensor.matmul(
                        ps_s[:, mc * 512 : (mc + 1) * 512],
                        qa[:, qt * P : (qt + 1) * P],
                        ka[:, m0 : m0 + 512],
                        start=True,
                        stop=True,
                    )
                nc.vector.reduce_max(
                    max2[:, 2 * qt + half2 : 2 * qt + half2 + 1],
                    ps_s[:],
                    axis=mybir.AxisListType.X,
                )

            def s_end(h):
                qa, max2 = q_tiles[h], max2_tiles[h]
                maxsb = mxp.tile([P, 8], f32, tag="mx", name=f"mx{h}")
                nc.vector.tensor_tensor(
                    out=maxsb[:],
                    in0=max2[:, 0:16:2],
                    in1=max2[:, 1:16:2],
                    op=mybir.AluOpType.max,
                )
                maxr = mxp.tile([P, 8], f32r, tag="mxr", name=f"mxr{h}")
                nc.vector.tensor_copy(maxr[:], maxsb[:])
                # scatter maxes into q-tilde row 64 (elem (p,qt) -> col qt*128+p)
                # via a transposing DRAM bounce
                nc.sync.dma_start(mstg_ap[h].rearrange("(a b) -> b a", b=P), maxr[:])
                nc.sync.dma_start(qa[D : D + 1, :], mstg_ap[h])

            av_tiles = {}

            def t_begin(h):
                av_tiles[h] = [
                    psAV.tile([D + 1, 512], f32, tag="av", name=f"av{h}_{i}")
                    for i in range(2)
                ]

            at_tiles = {}

            def t_st(h, mt):
                # both ncn S~^T matmuls + exps; independent of the AV pair of
                # mt-1 that issues after them (deferred one iteration so the
                # PE never waits on the Act engine)
                qa, ka = q_tiles[h], ka_tiles[h]
                ats = []
                for ncn in range(2):
                    ps_st = psST.tile([P, 512], f32, tag="st")
                    nc.tensor.matmul(
                        ps_st[:],
                        ka[:, mt * P : (mt + 1) * P],
                        qa[:, ncn * 512 : (ncn + 1) * 512],
                        start=True,
                        stop=True,
                    )
                    at = atp.tile([P, 512], bf16, tag="at")
                    nc.scalar.activation(at[:], ps_st[:], EXP, scale=8.0)
                    ats.append(at)
                at_tiles[(h, mt)] = ats

            def t_av(h, mt):
                ats = at_tiles.pop((h, mt))
                for ncn in range(2):
                    nc.tensor.matmul(
                        av_tiles[h][ncn][:],
                        vhat[:, mt, h, :],
                        ats[ncn][:],
                        start=(mt == 0),
                        stop=(mt == 15),
                    )

            def t_end(h):
                for ncn in range(2):
                    ps_av = av_tiles[h][ncn]
                    den = nrmp.tile([1, 512], f32, tag="den")
                    nc.scalar.copy(den[:], ps_av[D : D + 1, :])
                    rsb = nrmp.tile([1, 512], f32, tag="rec")
                    nc.vector.reciprocal_approx_fast(rsb[:], den[:])
                    rb = nrmp.tile([D, 512], f32, tag="rb")
                    nc.gpsimd.partition_broadcast(rb[:], rsb[:])
                    r0 = D * (h % 2)
                    nc.vector.tensor_mul(
                        outNT[r0 : r0 + D, h // 2, ncn * 512 : (ncn + 1) * 512],
                        ps_av[0:D, :],
                        rb[:],
                    )

            # software pipeline, two heads deep: during tail h the S-pass of
            # head h+2 runs one step per iteration, so its max-bounce DMA has
            # the whole of tail h+1 to land before tail h+2 needs the maxes.
            # AV of mt issues a full iteration after its exp started, so the
            # PE never stalls on the Act engine.
            s_begin(0)
            for i in range(16):
                s_step(0, i)
            s_end(0)
            # tail(0) carries the S-passes of BOTH heads 1 and 2 (two steps
            # per iteration; head 1 in the first half with its s_end mid-loop
            # so the max-bounce launches as soon as the reduces drain)
            for h in range(H):
                t_begin(h)
                if h == 0:
                    s_begin(1)
                    s_begin(2)
                elif h + 2 < H:
                    s_begin(h + 2)
                for mt in range(16):
                    t_st(h, mt)
                    if h == 0:
                        if mt < 8:
                            s_step(1, 2 * mt)
                            s_step(1, 2 * mt + 1)
                        else:
                            s_step(2, 2 * (mt - 8))
                            s_step(2, 2 * (mt - 8) + 1)
                        if mt == 7:
                            s_end(1)
                    elif h + 2 < H:
                        s_step(h + 2, mt)
                    if mt >= 1:
                        t_av(h, mt - 1)
                t_av(h, 15)
                t_end(h)
                if h == 0:
                    s_end(2)
                elif h + 2 < H:
                    s_end(h + 2)

        # ================= P3: output projection =================
        with (
            tc.tile_pool(name="ob", bufs=3) as obp,
            tc.tile_pool(name="psP", bufs=3, space="PSUM") as psP,
        ):
            for nt in range(8):
                ps = psP.tile([P, CD], f32, tag="pj")
                for kc in range(6):
                    for c0, cn in ((0, 512), (512, 256)):
                        nc.tensor.matmul(
                            ps[:, c0 : c0 + cn],
                            outNT[:, kc, nt * P : (nt + 1) * P],
                            wp_r[:, kc, c0 : c0 + cn],
                            start=(kc == 0),
                            stop=(kc == 5),
                        )
                osb = obp.tile([P, CD], f32, tag="ob")
                nc.scalar.copy(osb[:], ps[:])
                oeng = nc.sync if nt % 2 == 0 else nc.scalar
                oeng.dma_start(out_t[:, nt, :], osb[:])

    nc.compile()
    return nc


def _in_maps(x, w_qkv, w_proj):
    ident = np.eye(P, dtype=np.float32)
    crow = np.zeros((2, NK), np.float32)
    crow[1, :] = -1.0
    zeros = np.zeros((P - D - 1, NK), np.float32)
    maps = []
    for c in range(8):
        b, qh = c // 2, c % 2
        xb = np.roll(x[b], -qh * NQ, axis=0) if qh else x[b]
        maps.append(
            {
                "x": np.ascontiguousarray(xb, dtype=np.float32),
                "w_qkv": np.ascontiguousarray(w_qkv, dtype=np.float32),
                "w_proj": np.ascontiguousarray(w_proj, dtype=np.float32),
                "ident": ident,
                "crow": crow,
                "zeros": zeros,
            }
        )
    return maps


LAST = {}


def kernel(x, w_qkv, w_proj):
    import os

    from concourse import bass_utils

    if "nc" not in _CACHE:
        _CACHE["nc"] = _build()
    nc = _CACHE["nc"]
    kwargs = {}
    if os.environ.get("KERNEL_TRACE"):
        kwargs["trace"] = True
        if os.environ.get("KERNEL_TRACE_DIR"):
            kwargs["tmpdir"] = os.environ["KERNEL_TRACE_DIR"]
    res = bass_utils.run_bass_kernel_spmd(
        nc, _in_maps(x, w_qkv, w_proj), core_ids=list(range(8)), **kwargs
    )
    LAST["exec_time_ns"] = res.exec_time_ns
    out = np.empty((4, 2048, CD), np.float32)
    for c in range(8):
        b, qh = c // 2, c % 2
        out[b, qh * NQ : (qh + 1) * NQ] = res.results[c]["out"]
    return out



# revision 60
# speedup vs baseline: 1.0556x; 1.0297x over previous
"""Trainium2 Bass kernel for multi-head attention (B=4, N=2048, C=768, H=12).

Sharding: zero-collective data parallel across 8 NeuronCores. Core c handles
batch b=c//2 and query rows [(c%2)*1024, +1024). K/V are computed over all
2048 keys of the batch (softmax over keys is permutation invariant, so each
core receives its x with its own query rows rolled to the front). Pairs of
cores duplicate only the cheap K/V projection; there are no collectives.

Key performance facts (measured on hw): PE matmuls with contraction K=128
run at ~227ns/512 rows (2.4GHz); K<128 can silently run at HALF rate, so
the 65-row q/k tiles (64 dims + 1 shift lane) are zero-padded to 128
partitions (zeros DMA'd from DRAM; padding contributes 0 to the product).

Per-core pipeline (matmul compute dtype float32r ~ tf32; rel err ~2.8e-3
against the 2e-2 gate):
  P0: load x [2048,768] in 8 chunks alternating the SP/Act DMA queues,
      PE-transpose to xT [768,2048] (f32r rounding in the PSUM->SBUF copies).
  P1: QKV projections, w_qkv streamed in six 384-col chunks (bufs=2) so
      weight DMA+cast overlap the matmuls; kc-outer/ncn-inner loop order
      loads each weight tile once. q-tilde [128,1024] per head (row 64 =
      +row-max, rows 65+ zero), k-tilde (row 64 = -1) staged through DRAM,
      v-hat [128,16,h,65] bf16 (col 64 = ones so A^T.v-hat also yields the
      softmax denominators). w_proj is loaded/cast here too.
  P2: software-pipelined TWO heads deep. During the tail of head h, one
      S-pass step of head h+2 runs per iteration, so the row-max bounce DMA
      (PSUM row-max on DVE -> transposing DRAM bounce into q-tilde row 64)
      has the whole of tail h+1 to land. Tail iteration: S~^T pair (the
      65th lane applies -max) -> exp(8*x) on ScalarE -> bf16 A^T; the AV
      pair of mt-1 issues after the ST pair of mt so the in-order PE queue
      never waits on the Act engine. Normalization: reciprocal_approx_fast
      (denominators are in [1, 2048] -> safe) + gpsimd partition_broadcast
      + DVE multiply into outNT [768,1024] f32r.
  P3: projection out = outNT.T @ w_proj (outNT is directly the lhsT),
      kc-outer loop order, DMA out [1024,768].
"""

import numpy as np

NQ = 1024  # queries per core
NK = 2048  # keys per core
CD = 768
H = 12
D = 64
P = 128

_CACHE = {}


def _build():
    from contextlib import ExitStack

    import concourse.bacc as bacc
    import concourse.mybir as mybir
    import concourse.tile as tile

    f32 = mybir.dt.float32
    f32r = mybir.dt.float32r
    bf16 = mybir.dt.bfloat16
    EXP = mybir.ActivationFunctionType.Exp
    LOG = mybir.ActivationFunctionType.Ln

    nc = bacc.Bacc("TRN2", target_bir_lowering=False, debug=False, num_devices=8)

    x_ap = nc.dram_tensor("x", [NK, CD], f32, kind="ExternalInput").ap()
    wqkv_ap = nc.dram_tensor("w_qkv", [CD, 3 * CD], f32, kind="ExternalInput").ap()
    wproj_ap = nc.dram_tensor("w_proj", [CD, CD], f32, kind="ExternalInput").ap()
    ident_ap = nc.dram_tensor("ident", [P, P], f32, kind="ExternalInput").ap()
    # crow[0,:] = 0.0 (q-tilde row 64 init), crow[1,:] = -1.0 (k-tilde row 64)
    crow_ap = nc.dram_tensor("crow", [2, NK], f32r, kind="ExternalInput").ap()
    out_ap = nc.dram_tensor("out", [NQ, CD], f32, kind="ExternalOutput").ap()

    # zero padding rows 65..127 of q/k tiles: K=128 matmuls run 2x faster
    # than K=65 on the PE (empirically; K<128 can drop to half rate)
    zeros_ap = nc.dram_tensor("zeros", [P - D - 1, NK], f32r, kind="ExternalInput").ap()

    kstg_ap = nc.dram_tensor("kstg", [H, D + 1, NK], f32r).ap()
    mstg_ap = nc.dram_tensor("mstg", [H, NQ], f32r).ap()

    x_t = x_ap.rearrange("(t p) c -> p t c", p=P)  # [128, 16, 768]
    wqkv_t = wqkv_ap.rearrange("(a p) n -> p a n", p=P)  # [128, 6, 2304]
    wproj_t = wproj_ap.rearrange("(a p) n -> p a n", p=P)  # [128, 6, 768]
    out_t = out_ap.rearrange("(t p) c -> p t c", p=P)  # [128, 8, 768]

    with tile.TileContext(nc) as tc, ExitStack() as ctx:
        # ---- persistent pools ----
        pers = ctx.enter_context(tc.tile_pool(name="pers", bufs=1))
        # padded so downstream pools (xT, q-tiles) start 4KB-aligned — matmuls
        # whose moving operand is misaligned from 1KB run ~57% slower
        ident_sb = pers.tile([P, P], f32, tag="ident", padded_shape=[P, 1024])
        nc.sync.dma_start(ident_sb[:], ident_ap)

        qpool = ctx.enter_context(tc.tile_pool(name="qt", bufs=1))
        q_tiles = [
            qpool.tile([P, NQ], f32r, tag=f"q{h}", name=f"q{h}") for h in range(H)
        ]

        vpool = ctx.enter_context(tc.tile_pool(name="vhat", bufs=1))
        vhat = vpool.tile([P, 16, H, D + 1], bf16, tag="vhat")

        wpp = ctx.enter_context(tc.tile_pool(name="wp", bufs=1))
        wp_r = wpp.tile([P, 6, CD], f32r, tag="wp")

        from contextlib import ExitStack as _ES

        ctx_xT = _ES()
        xT_pool = ctx_xT.enter_context(tc.tile_pool(name="xT", bufs=1))
        xT = xT_pool.tile([P, 6, NK], f32r, tag="xT")  # [C-chunk part, kc, row]

        # ================= P0: load x, transpose =================
        with (
            tc.tile_pool(name="xn", bufs=1) as xnp,
            tc.tile_pool(name="ps0", bufs=4, space="PSUM") as ps0,
        ):
            xn = xnp.tile([P, 16, CD], f32, tag="xn")
            for tq in range(16):
                eng = nc.sync if tq % 2 == 0 else nc.scalar
                eng.dma_start(xn[:, tq : tq + 1, :], x_t[:, tq : tq + 1, :])
            # q-tile zero padding, queued after the x chunks; only needed
            # before P2's S-pass
            for h in range(H):
                eng = nc.sync if h % 2 == 0 else nc.scalar
                eng.dma_start(q_tiles[h][D + 1 : P, :], zeros_ap[:, 0:NQ])
            for t in range(16):
                for kc in range(6):
                    pst = ps0.tile([P, P], f32, tag="tr")
                    nc.tensor.transpose(
                        pst[:], xn[:, t, kc * P : (kc + 1) * P], ident_sb[:]
                    )
                    nc.vector.tensor_copy(
                        xT[:, kc, t * P : (t + 1) * P], pst[:]
                    )  # f32 -> f32r round

        # ================= P1: QKV projections =================
        # w_qkv streamed in six 384-col chunks (bufs=2) so weight DMA+cast of
        # chunk c+1 overlaps the matmuls of chunk c
        def qk_col_block(j, wq_r, jl, is_q):
            """One 128-col block (heads 2j, 2j+1) of the q or k projection."""
            nch = 2 if is_q else 4
            if not is_q:
                dsts = [
                    kstg_pool.tile(
                        [D + 1, NK], f32r, tag="kst", name=f"kst{2 * j + i}"
                    )
                    for i in range(2)
                ]
            else:
                dsts = [q_tiles[2 * j], q_tiles[2 * j + 1]]
            # kc outer / ncn inner: each weight tile is loaded once and used
            # for all ncn chunks (the PE skips nothing, but redundant
            # LDWEIGHTS pressure goes away)
            ps_list = [ps1.tile([P, 512], f32, tag="qk", name=f"qk{_n}") for _n in range(nch)]
            for kc in range(6):
                for ncn in range(nch):
                    nc.tensor.matmul(
                        ps_list[ncn][:],
                        wq_r[:, kc, jl * P : (jl + 1) * P],
                        xT[:, kc, ncn * 512 : (ncn + 1) * 512],
                        start=(kc == 0),
                        stop=(kc == 5),
                    )
            for ncn in range(nch):
                # split the PSUM->SBUF copies between Act and DVE: both have
                # slack in P1 and the copies gate the k-staging writes
                nc.scalar.copy(
                    dsts[0][0:D, ncn * 512 : (ncn + 1) * 512],
                    ps_list[ncn][0:D, :],
                )
                nc.vector.tensor_copy(
                    dsts[1][0:D, ncn * 512 : (ncn + 1) * 512],
                    ps_list[ncn][D : 2 * D, :],
                )
            for i in range(2):
                if is_q:
                    nc.sync.dma_start(dsts[i][D : D + 1, 0:NQ], crow_ap[0, 0:NQ])
                else:
                    nc.sync.dma_start(dsts[i][D : D + 1, :], crow_ap[1, :])
                    nc.sync.dma_start(kstg_ap[2 * j + i], dsts[i][:])

        with (
            tc.tile_pool(name="wq", bufs=2) as wqp,
            tc.tile_pool(name="wtmp", bufs=3) as wtp,
            tc.tile_pool(name="kstg", bufs=4) as kstg_pool,
            tc.tile_pool(name="ps1", bufs=4, space="PSUM") as ps1,
            tc.tile_pool(name="ps1v", bufs=4, space="PSUM") as ps1v,
        ):
            def load_chunk(c):
                wq_r = wqp.tile([P, 6, 384], f32r, tag="wq")
                for a in range(6):
                    wtmp = wtp.tile([P, 384], f32, tag="wt")
                    eng = nc.sync if a % 2 == 0 else nc.scalar
                    eng.dma_start(wtmp[:], wqkv_t[:, a, 384 * c : 384 * (c + 1)])
                    nc.vector.tensor_copy(wq_r[:, a, :], wtmp[:])
                return wq_r

            wq_r0 = load_chunk(0)
            # w_proj load queued after the first chunk so it never gates P1
            for a in range(6):
                wtmp3t = wtp.tile([P, CD], f32, tag="wt3")
                eng = nc.sync if a % 2 == 0 else nc.scalar
                eng.dma_start(wtmp3t[:], wproj_t[:, a, :])
                nc.vector.tensor_copy(wp_r[:, a, :], wtmp3t[:])
            for c in range(2):  # q cols 0..767
                wq_r = wq_r0 if c == 0 else load_chunk(c)
                for jl in range(3):
                    qk_col_block(3 * c + jl, wq_r, jl, is_q=True)
            for c in range(2, 4):  # k cols 768..1535
                wq_r = load_chunk(c)
                for jl in range(3):
                    qk_col_block(3 * (c - 2) + jl, wq_r, jl, is_q=False)
            for c in range(4, 6):  # v cols 1536..2303
                wq_r = load_chunk(c)
                vc = c - 4
                for t in range(16):
                    ps = ps1v.tile([P, 384], f32, tag="vp")
                    for kc in range(6):
                        nc.tensor.matmul(
                            ps[:],
                            xT[:, kc, t * P : (t + 1) * P],
                            wq_r[:, kc, :],
                            start=(kc == 0),
                            stop=(kc == 5),
                        )
                    nc.vector.tensor_copy(
                        vhat[:, t, 6 * vc : 6 * (vc + 1), 0:D],
                        ps[:].rearrange("p (h d) -> p h d", d=D),
                    )
            nc.gpsimd.memset(vhat[:, :, :, D], 1.0)
        ctx_xT.close()  # xT dead after P1 — free 48KB/partition for P2

        # ================= P2: attention per head =================
        opool = ctx.enter_context(tc.tile_pool(name="outNT", bufs=1))
        outNT = opool.tile([P, 6, NQ], f32r, tag="outNT")
        with (
            tc.tile_pool(name="ka", bufs=3) as kap,
            tc.tile_pool(name="mx", bufs=2) as mxp,
            tc.tile_pool(name="at", bufs=4) as atp,
            tc.tile_pool(name="nrm", bufs=2) as nrmp,
            tc.tile_pool(name="scr", bufs=2) as scrp,
            tc.tile_pool(name="psS", bufs=2, space="PSUM") as psS,
            tc.tile_pool(name="psST", bufs=2, space="PSUM") as psST,
            tc.tile_pool(name="psAV", bufs=2, space="PSUM") as psAV,
        ):
            ka_tiles = {}
            max2_tiles = {}

            # zero rows 65..127 of the three rotating ka buffers once; later
            # generations only DMA rows 0..64, the pad rows stay zero forever
            for b in range(3):
                kaz = kap.tile([P, NK], f32r, tag="ka", name=f"kaz{b}")
                eng = nc.sync if b % 2 == 0 else nc.scalar
                eng.dma_start(kaz[D + 1 : P, :], zeros_ap)

            def s_begin(h):
                ka = kap.tile([P, NK], f32r, tag="ka", name=f"ka{h}")
                ka_tiles[h] = ka
                nc.sync.dma_start(ka[0 : D + 1, :], kstg_ap[h])
                max2_tiles[h] = mxp.tile([P, 16], f32, tag="mx2", name=f"mx2{h}")

            def s_step(h, i):
                # one [128,1024] S tile: queries qt=i//2, keys half2=i%2.
                # tensor_tensor_reduce fuses pairwise-max of the two 512-col
                # halves with the row-max reduction in a single DVE pass
                qa, ka, max2 = q_tiles[h], ka_tiles[h], max2_tiles[h]
                qt, half2 = i // 2, i % 2
                ps_s = psS.tile([P, NK // 2], f32, tag="S")
                for mc in range(2):
                    m0 = half2 * 1024 + mc * 512
                    nc.tensor.matmul(
                        ps_s[:, mc * 512 : (mc + 1) * 512],
                        qa[:, qt * P : (qt + 1) * P],
                        ka[:, m0 : m0 + 512],
                        start=True,
                        stop=True,
                    )
                nc.vector.reduce_max(
                    max2[:, 2 * qt + half2 : 2 * qt + half2 + 1],
                    ps_s[:],
                    axis=mybir.AxisListType.X,
                )

            def s_end(h):
                qa, max2 = q_tiles[h], max2_tiles[h]
                maxsb = mxp.tile([P, 8], f32, tag="mx", name=f"mx{h}")
                nc.vector.tensor_tensor(
                    out=maxsb[:],
                    in0=max2[:, 0:16:2],
                    in1=max2[:, 1:16:2],
                    op=mybir.AluOpType.max,
                )
                maxr = mxp.tile([P, 8], f32r, tag="mxr", name=f"mxr{h}")
                nc.vector.tensor_copy(maxr[:], maxsb[:])
                # scatter maxes into q-tilde row 64 (elem (p,qt) -> col qt*128+p)
                # via a transposing DRAM bounce
                nc.sync.dma_start(mstg_ap[h].rearrange("(a b) -> b a", b=P), maxr[:])
                nc.sync.dma_start(qa[D : D + 1, :], mstg_ap[h])

            av_tiles = {}

            def t_begin(h):
                av_tiles[h] = [
                    psAV.tile([D + 1, 512], f32, tag="av", name=f"av{h}_{i}")
                    for i in range(2)
                ]

            at_tiles = {}

            def t_st(h, mt):
                # both ncn S~^T matmuls + exps; independent of the AV pair of
                # mt-1 that issues after them (deferred one iteration so the
                # PE never waits on the Act engine)
                qa, ka = q_tiles[h], ka_tiles[h]
                ats = []
                for ncn in range(2):
                    ps_st = psST.tile([P, 512], f32, tag="st")
                    nc.tensor.matmul(
                        ps_st[:],
                        ka[:, mt * P : (mt + 1) * P],
                        qa[:, ncn * 512 : (ncn + 1) * 512],
                        start=True,
                        stop=True,
                    )
                    at = atp.tile([P, 512], bf16, tag="at")
                    nc.scalar.activation(at[:], ps_st[:], EXP, scale=8.0)
                    ats.append(at)
                at_tiles[(h, mt)] = ats

            def t_av(h, mt):
                ats = at_tiles.pop((h, mt))
                for ncn in range(2):
                    nc.tensor.matmul(
                        av_tiles[h][ncn][:],
                        vhat[:, mt, h, :],
                        ats[ncn][:],
                        start=(mt == 0),
                        stop=(mt == 15),
                    )

            def t_end(h):
                for ncn in range(2):
                    ps_av = av_tiles[h][ncn]
                    den = nrmp.tile([1, 512], f32, tag="den")
                    nc.scalar.copy(den[:], ps_av[D : D + 1, :])
                    rsb = nrmp.tile([1, 512], f32, tag="rec")
                    nc.vector.reciprocal_approx_fast(rsb[:], den[:])
                    rb = nrmp.tile([D, 512], f32, tag="rb")
                    nc.gpsimd.partition_broadcast(rb[:], rsb[:])
                    r0 = D * (h % 2)
                    nc.vector.tensor_mul(
                        outNT[r0 : r0 + D, h // 2, ncn * 512 : (ncn + 1) * 512],
                        ps_av[0:D, :],
                        rb[:],
                    )

            # software pipeline, two heads deep: during tail h the S-pass of
            # head h+2 runs one step per iteration, so its max-bounce DMA has
            # the whole of tail h+1 to land before tail h+2 needs the maxes.
            # AV of mt issues a full iteration after its exp started, so the
            # PE never stalls on the Act engine.
            s_begin(0)
            for i in range(16):
                s_step(0, i)
            s_end(0)
            # tail(0) carries the S-passes of BOTH heads 1 and 2 (two steps
            # per iteration; head 1 in the first half with its s_end mid-loop
            # so the max-bounce launches as soon as the reduces drain)
            for h in range(H):
                t_begin(h)
                if h == 0:
                    s_begin(1)
                    s_begin(2)
                elif h + 2 < H:
                    s_begin(h + 2)
                for mt in range(16):
                    t_st(h, mt)
                    if h == 0:
                        if mt < 8:
                            s_step(1, 2 * mt)
                            s_step(1, 2 * mt + 1)
                        else:
                            s_step(2, 2 * (mt - 8))
                            s_step(2, 2 * (mt - 8) + 1)
                        if mt == 7:
                            s_end(1)
                    elif h + 2 < H:
                        s_step(h + 2, mt)
                    if mt >= 1:
                        t_av(h, mt - 1)
                t_av(h, 15)
                t_end(h)
                if h == 0:
                    s_end(2)
                elif h + 2 < H:
                    s_end(h + 2)

        # ================= P3: output projection =================
        with (
            tc.tile_pool(name="ob", bufs=3) as obp,
            tc.tile_pool(name="psP", bufs=3, space="PSUM") as psP,
        ):
            for nt in range(8):
                ps = psP.tile([P, CD], f32, tag="pj")
                for kc in range(6):
                    for c0, cn in ((0, 512), (512, 256)):
                        nc.tensor.matmul(
                            ps[:, c0 : c0 + cn],
                            outNT[:, kc, nt * P : (nt + 1) * P],
                            wp_r[:, kc, c0 : c0 + cn],
                            start=(kc == 0),
                            stop=(kc == 5),
                        )
                osb = obp.tile([P, CD], f32, tag="ob")
                nc.scalar.copy(osb[:], ps[:])
                oeng = nc.sync if nt % 2 == 0 else nc.scalar
                oeng.dma_start(out_t[:, nt, :], osb[:])

    nc.compile()
    return nc


def _in_maps(x, w_qkv, w_proj):
    ident = np.eye(P, dtype=np.float32)
    crow = np.zeros((2, NK), np.float32)
    crow[1, :] = -1.0
    zeros = np.zeros((P - D - 1, NK), np.float32)
    maps = []
    for c in range(8):
        b, qh = c // 2, c % 2
        xb = np.roll(x[b], -qh * NQ, axis=0) if qh else x[b]
        maps.append(
            {
                "x": np.ascontiguousarray(xb, dtype=np.float32),
                "w_qkv": np.ascontiguousarray(w_qkv, dtype=np.float32),
                "w_proj": np.ascontiguousarray(w_proj, dtype=np.float32),
                "ident": ident,
                "crow": crow,
                "zeros": zeros,
            }
        )
    return maps


LAST = {}


def kernel(x, w_qkv, w_proj):
    import os

    from concourse import bass_utils

    if "nc" not in _CACHE:
        _CACHE["nc"] = _build()
    nc = _CACHE["nc"]
    kwargs = {}
    if os.environ.get("KERNEL_TRACE"):
        kwargs["trace"] = True
        if os.environ.get("KERNEL_TRACE_DIR"):
            kwargs["tmpdir"] = os.environ["KERNEL_TRACE_DIR"]
    res = bass_utils.run_bass_kernel_spmd(
        nc, _in_maps(x, w_qkv, w_proj), core_ids=list(range(8)), **kwargs
    )
    LAST["exec_time_ns"] = res.exec_time_ns
    out = np.empty((4, 2048, CD), np.float32)
    for c in range(8):
        b, qh = c // 2, c % 2
        out[b, qh * NQ : (qh + 1) * NQ] = res.results[c]["out"]
    return out

